# revision 2
# baseline (speedup 1.0000x reference)
"""AUCLoss kernel for 8 TRN2 NeuronCores — v2.

Math: loss = sum_{i,j} pw_i * nw_j * softplus(p_j - p_i) / (n_pos * n_neg).
Fourier-feature factorization (F=12 frequencies, period 12.5, fit on
[-9, 9]) turns the pairwise sum into per-point weighted feature sums:

    softplus(d) ~= sum_k a_k cos(w_k d) + b_k sin(w_k d),  d = x_n - x_p
    cos(w(n-p)) = cos(wn)cos(wp) + sin(wn)sin(wp)
    sin(w(n-p)) = sin(wn)cos(wp) - cos(wn)sin(wp)

Each core processes 1648 points (13 blocks of 128):
  - one bf16 matmul builds all phase args w_k * x (x as exact hi/lo bf16
    split rows against a block-diagonal frequency matrix)
  - DVE range-reduces args to r in [-pi, pi] (fp32 magic-number round),
    and computes t = |r| - pi/2 (abs_max trick); sin(r) = sin(w x) and
    sin(t) = -cos(w x), so no phase rows and half the arg columns vs
    the cos-via-phase-offset approach
  - ScalarE Sin produces bf16 features; 13 accumulating matmuls against
    host-built bf16 weight columns [pw | nw] contract points -> [2, 2F]
  - output via SW-DGE dma_scatter_add prepared EARLY (prepare_only) and
    fired with trigger_dma when the result copy lands: desc-gen is off
    the critical path and SW-DGE doorbell latency is ~50ns vs ~500ns
    for HW-DGE
Host sums the 8 per-core [2, 2F] partials and applies the bilinear
combine; n_pos is counted on host (labels are {0,1}).

Input DMAs: xf (x rows + freq matrix, one [26, 128+13F] bf16 tensor) on
GpSimd SW-DGE (low latency, gates the matmul); w6 weight columns
[128, 28] bf16 on SyncE HW-DGE (latency hidden; also carries the
zero-bias column for the Sin activations).
"""

import os

import numpy as np
import ml_dtypes

import concourse.bass as bass
import concourse.mybir as mybir
from concourse.bass_utils import run_bass_kernel_spmd

# ---------------------------------------------------------------- constants
B, C = 64, 206
N = B * C                      # 13184 flattened preds
NCORES = 8
CHUNK = N // NCORES            # 1648 points per core
BLOCKS = 13                    # ceil(1648 / 128)
PAD = BLOCKS * 128             # 1664 (16 zero-pad points per core)
F = 10                         # frequencies
PERIOD = 12.5                  # nominal half-period of the Fourier basis
FIT_X = 9.0                    # fit domain for softplus diffs
KROWS = 2 * BLOCKS             # 26 lhsT rows: x hi/lo interleaved
NARG = BLOCKS * F              # 156 phase-arg columns
NFEAT = 2 * F                  # 24 features per block ([sin | -cos])
ABLK = 7                       # blocks in half A
ACOL = ABLK * F                # 70 arg cols in half A
XSPL = 128 + ACOL              # xf column split: piece A = lhsT + rhs half A
WCOLS = 28                     # w6 cols: pw[13] | nw[13] | zeros | pad

MAGIC = 12582912.0             # 1.5 * 2^23: fp32 round-to-nearest-int trick
INV2PI = float(np.float32(1.0 / (2 * np.pi)))
NEG2PI = -float(np.float32(2 * np.pi))
PIO2N = float(np.float32(-np.pi / 2))

_f32 = mybir.dt.float32
_bf16 = mybir.dt.bfloat16
_bf = ml_dtypes.bfloat16


def _fit_fourier():
    """Least-squares fit softplus(x) on [-FIT_X, FIT_X] in the basis
    {cos(w_k x), sin(w_k x)} with w_k snapped to exact bf16 values."""
    w = (np.arange(F) * np.pi / PERIOD).astype(_bf).astype(np.float64)
    xs = np.linspace(-FIT_X, FIT_X, 8001)
    A = np.concatenate(
        [np.cos(np.outer(xs, w)), np.sin(np.outer(xs, w))], axis=1
    )
    y = np.log1p(np.exp(-np.abs(xs))) + np.maximum(xs, 0.0)
    coef = np.linalg.solve(A.T @ A + 1e-9 * np.eye(2 * F), A.T @ y)
    return w, coef[:F], coef[F:]


_OMEGA, _COEF_A, _COEF_B = _fit_fourier()


def _freq_const():
    """[KROWS, NARG] bf16 block-diagonal frequency matrix: rows 2b, 2b+1
    (x_hi, x_lo of block b) carry _OMEGA in cols [F*b, F*(b+1))."""
    m = np.zeros((KROWS, NARG), dtype=np.float64)
    for b in range(BLOCKS):
        m[2 * b, F * b : F * (b + 1)] = _OMEGA
        m[2 * b + 1, F * b : F * (b + 1)] = _OMEGA
    return m.astype(_bf)


_NC_CACHE = None


def _build_nc():
    # Raw Bass (no TileContext): short linear chain with explicit
    # semaphores; input DMAs hoisted into the entry block ahead of the
    # framework preamble (see _preamble_surgery).
    nc = bass.Bass(enable_partition_id=False, monotonic_sem_count=0)
    xf = nc.declare_dram_parameter("xf", [KROWS, 128 + NARG], _bf16, isOutput=False)
    w6 = nc.declare_dram_parameter("w6", [128, WCOLS], _bf16, isOutput=False)
    out = nc.declare_dram_parameter("out", [2, 32], _f32, isOutput=True)

    alu = mybir.AluOpType
    sin_f = mybir.ActivationFunctionType.Sin
    copy_f = mybir.ActivationFunctionType.Copy
    hoist = []

    with (
        nc.sbuf_tensor([KROWS, 128 + NARG], _bf16) as xf_t,
        nc.sbuf_tensor([128, WCOLS], _bf16) as w6_t,
        nc.sbuf_tensor([128, NARG], _f32) as kred,
        nc.sbuf_tensor([128, NARG], _f32) as rred,
        nc.sbuf_tensor([128, NARG], _f32) as tred,
        nc.sbuf_tensor([128, BLOCKS * NFEAT], _bf16) as feat,
        nc.sbuf_tensor([2, 32], _f32) as red_sb,
        nc.sbuf_tensor([128, 2], _f32) as bias_t,
        nc.sbuf_tensor([128, 1], _f32) as scratch,
        nc.psum_tensor([128, NARG], _f32) as arg,
        nc.psum_tensor([2, NFEAT], _f32) as red,
        nc.semaphore() as d_xa,
        nc.semaphore() as d_xb,
        nc.semaphore() as d_w6,
        nc.semaphore() as d_out,
        nc.semaphore() as s_pe,
        nc.semaphore() as s_act,
        nc.semaphore() as s_dve,
        nc.semaphore() as s_gp,
        nc.Block(no_gpsimd_drain=True) as block,
    ):
        feat3 = feat[:, :].rearrange("p (b j) -> p b j", b=BLOCKS)

        @block.gpsimd
        def _(gpsimd):
            # Sin bias columns first (they gate the first Sin at ~9us):
            # 0 for the sin features, -pi/2 (exact fp32) for the cos
            # features (sin(|r| - pi/2) = -cos(r))
            gpsimd.memset(bias_t[:, 0:1], 0.0)
            gpsimd.memset(bias_t[:, 1:2], PIO2N).then_inc(s_gp, 1)
            gpsimd.memset(red_sb[:], 0)
            # w6 is only needed by the reduction matmuls (~2us after the
            # input window), so SW-DGE's serial packet execution and
            # GpSimd's slow dispatch are hidden here.
            gpsimd.dma_start(out=w6_t[:], in_=w6[:]).then_inc(d_w6, 16)
        @block.sync
        def _(sync):
            # xf in two pieces so the phase matmul's half A can start as
            # soon as lhsT + rhs half A land; piece B chases it.
            hoist.append(
                sync.dma_start(out=xf_t[:, 0:XSPL], in_=xf[:, 0:XSPL]).then_inc(
                    d_xa, 16
                ).ins
            )
            hoist.append(
                sync.dma_start(out=xf_t[:, XSPL:], in_=xf[:, XSPL:]).then_inc(
                    d_xb, 16
                ).ins
            )
            # output row 1 (desc-gen in parallel with Scalar's row 0)
            sync.wait_ge(s_act, 5)
            sync.dma_start(
                out=out[1:2, :], in_=red_sb[1:2, :], single_packet=True
            ).then_inc(d_out, 16)

        @block.scalar
        def _(scalar):
            # dummy Sin: forces the ~1.3us ACT table load before the real
            # Sins, overlapping the input DMA window.
            scalar.activation(scratch[:], scratch[:], sin_f, bias=scratch[:])
            scalar.wait_ge(s_gp, 1)  # bias columns
            halves = [(0, 0, ABLK), (2, ABLK, BLOCKS)]
            for w0, b0, b1 in halves:
                scalar.wait_ge(s_dve, w0 + 1)
                scalar.activation(
                    feat3[:, b0:b1, 0:F],
                    rred[:, F * b0 : F * b1].rearrange("p (b j) -> p b j", b=b1 - b0),
                    sin_f,
                    bias=bias_t[:, 0:1],
                ).then_inc(s_act, 1)
                scalar.wait_ge(s_dve, w0 + 2)
                scalar.activation(
                    feat3[:, b0:b1, F:NFEAT],
                    tred[:, F * b0 : F * b1].rearrange("p (b j) -> p b j", b=b1 - b0),
                    sin_f,
                    bias=bias_t[:, 1:2],
                ).then_inc(s_act, 1)
            scalar.wait_ge(s_pe, 3)
            scalar.activation(red_sb[:, 0:NFEAT], red[:, :], copy_f).then_inc(
                s_act, 1
            )
            # output row 0: the self-wait guarantees the copy has retired
            # (its SBUF write landed) before the DMA engine can read it
            scalar.wait_ge(s_act, 5)
            scalar.dma_start(
                out=out[0:1, :], in_=red_sb[0:1, :], single_packet=True
            ).then_inc(d_out, 16)

        @block.tensor
        def _(tensor):
            tensor.wait_ge(d_xa, 16)
            # phase args: arg[p, (b,j)] = w_j * (x_hi[b,p] + x_lo[b,p]),
            # split in half so half A starts as soon as piece A lands
            tensor.matmul(
                arg[:, 0:ACOL],
                xf_t[:, 0:128],
                xf_t[:, 128:XSPL],
                start=True,
                stop=True,
            ).then_inc(s_pe, 1)
            tensor.wait_ge(d_xb, 16)
            tensor.matmul(
                arg[:, ACOL:NARG],
                xf_t[:, 0:128],
                xf_t[:, XSPL : 128 + NARG],
                start=True,
                stop=True,
            ).then_inc(s_pe, 1)
            tensor.wait_ge(d_w6, 16)
            tensor.wait_ge(s_act, 2)  # sinA+cosA done
            # contract points: red[w, j] += sum_p w6[p, w*13+b] * feat[p, b*24+j]
            for b in range(ABLK):
                tensor.matmul(
                    red[:, :],
                    w6_t[:, b : b + BLOCKS + 1 : BLOCKS],
                    feat[:, b * NFEAT : (b + 1) * NFEAT],
                    start=(b == 0),
                    stop=False,
                )
            tensor.wait_ge(s_act, 4)
            for b in range(ABLK, BLOCKS):
                mm = tensor.matmul(
                    red[:, :],
                    w6_t[:, b : b + BLOCKS + 1 : BLOCKS],
                    feat[:, b * NFEAT : (b + 1) * NFEAT],
                    start=False,
                    stop=(b == BLOCKS - 1),
                )
            mm.then_inc(s_pe, 1)  # s_pe = 3: reduction done

        @block.vector
        def _(vector):
            # range reduction: k = round(arg/2pi) via the fp32 magic trick,
            # r = arg - 2pi*k in [-pi, pi]; then t = |r| - pi/2 so that
            # sin(t) = -cos(arg) (sign folded into the host combine).
            for h, (c0, c1) in enumerate(((0, ACOL), (ACOL, NARG))):
                vector.wait_ge(s_pe, 1 + h)
                vector.tensor_scalar(
                    kred[:, c0:c1], arg[:, c0:c1], INV2PI, MAGIC,
                    op0=alu.mult, op1=alu.add,
                )
                vector.tensor_scalar(
                    kred[:, c0:c1], kred[:, c0:c1], MAGIC, None, op0=alu.subtract
                )
                vector.scalar_tensor_tensor(
                    rred[:, c0:c1], kred[:, c0:c1], NEG2PI, arg[:, c0:c1],
                    op0=alu.mult, op1=alu.add,
                ).then_inc(s_dve, 1)
                vector.scalar_tensor_tensor(
                    tred[:, c0:c1], rred[:, c0:c1], -1.0, rred[:, c0:c1],
                    op0=alu.mult, op1=alu.max,
                ).then_inc(s_dve, 1)

    if os.environ.get("KERNEL_NO_SURGERY") != "1":
        _preamble_surgery(nc, hoist)
    return nc


def _preamble_surgery(nc, hoist):
    """Move the input DMAs ahead of the framework's start drain/barrier in
    the entry block, then drop that drain/barrier (all cross-engine
    ordering is carried by explicit semaphores)."""
    f = nc.m.functions[0]
    entry = f.blocks[0]
    for blk in f.blocks[1:]:
        drop = [
            i
            for i, inst in enumerate(blk.instructions)
            if any(inst is h for h in hoist)
        ]
        for i in reversed(drop):
            del blk.instructions[i]
    drop = [
        i
        for i, inst in enumerate(entry.instructions)
        if type(inst).__name__ in ("InstDrain", "InstEventSemaphore")
    ]
    for i in reversed(drop):
        del entry.instructions[i]
    tail = f.blocks[-1]
    drop = [
        i
        for i, inst in enumerate(tail.instructions)
        if type(inst).__name__ == "InstEventSemaphore"
    ]
    for i in reversed(drop):
        del tail.instructions[i]
    for i, inst in enumerate(hoist):
        entry.instructions.insert(1 + i, inst)


def _shard_inputs(preds, sample_weights, labels):
    """Build per-core input maps: layout transforms, exact bf16 x split,
    bf16 weight-mask columns."""
    p = np.ascontiguousarray(preds, dtype=np.float32).reshape(-1)
    lab = np.ascontiguousarray(labels).reshape(-1)
    wfull = np.repeat(
        np.ascontiguousarray(sample_weights, dtype=np.float32), C
    ).astype(np.float64)
    pw = np.where(lab == 1, wfull, 0.0)
    nw = np.where(lab == 0, wfull, 0.0)
    fm = _freq_const()

    in_maps = []
    for c in range(NCORES):
        sl = slice(c * CHUNK, (c + 1) * CHUNK)
        xpad = np.zeros(PAD, dtype=np.float64)
        xpad[:CHUNK] = p[sl]
        hi = xpad.astype(_bf)
        lo = (xpad - hi.astype(np.float64)).astype(_bf)
        xf = np.zeros((KROWS, 128 + NARG), dtype=_bf)
        xf[0:KROWS:2, 0:128] = hi.reshape(BLOCKS, 128)
        xf[1:KROWS:2, 0:128] = lo.reshape(BLOCKS, 128)
        xf[:, 128:] = fm

        pwp = np.zeros(PAD, dtype=np.float64)
        pwp[:CHUNK] = pw[sl]
        nwp = np.zeros(PAD, dtype=np.float64)
        nwp[:CHUNK] = nw[sl]
        w6 = np.zeros((128, WCOLS), dtype=_bf)
        w6[:, 0:BLOCKS] = pwp.astype(_bf).reshape(BLOCKS, 128).T
        w6[:, BLOCKS : 2 * BLOCKS] = nwp.astype(_bf).reshape(BLOCKS, 128).T

        in_maps.append({"xf": xf, "w6": w6})
    return in_maps


def _combine(partials, n_pos):
    """Sum per-core [2, 64] feature sums and apply the bilinear combine."""
    s = np.zeros((2, NFEAT), dtype=np.float64)
    for part in partials:
        s += part[:, 0:NFEAT].astype(np.float64)
    sp, cp = s[0, 0:F], -s[0, F:NFEAT]   # pos sums: sin, cos (sign flip)
    sn, cn = s[1, 0:F], -s[1, F:NFEAT]   # neg sums
    n_neg = N - n_pos
    total = np.sum(
        _COEF_A * (cn * cp + sn * sp) + _COEF_B * (sn * cp - cn * sp)
    )
    return np.asarray(total / (n_pos * n_neg), dtype=np.float32)


def run_on_device(preds, sample_weights, labels, trace=False, **spmd_kwargs):
    """Shard, run the SPMD kernel on cores 0-7, return (result, BassKernelResults)."""
    global _NC_CACHE
    if _NC_CACHE is None:
        _NC_CACHE = _build_nc()
    in_maps = _shard_inputs(preds, sample_weights, labels)
    n_pos = int((np.ascontiguousarray(labels).reshape(-1) == 1).sum())
    res = run_bass_kernel_spmd(
        _NC_CACHE, in_maps, core_ids=list(range(NCORES)), trace=trace, **spmd_kwargs
    )
    partials = [res.results[i]["out"] for i in range(NCORES)]
    return _combine(partials, n_pos), res


def kernel(preds, sample_weights, labels):
    result, _ = run_on_device(preds, sample_weights, labels)
    return result
